# revision 8
# baseline (speedup 1.0000x reference)
"""Trainium2 Bass kernel for nn_ExtractorMLP (gather + 3-layer edge MLP), v2.

Strategy (v2, instruction-count-driven)
---------------------------------------
This axon runtime is instruction-dispatch-bound (~85us/instruction on the
busiest engine stream; data movement is comparatively free), so v2 minimizes
instructions on the critical (tensor) stream:

1. bf16 on-chip. emb is shipped as TWO bf16 tables with 256B rows:
   embgL[i] = [bf16(emb[i]) | zeros(64)], embgH[i] = [zeros(64) | bf16(emb[i])].
   A transpose-mode dma_gather (elem_size=128, 16-bit granularity) writes
   unit u of the gathered row to partition u, free position = edge ordinal:
   the gather itself produces the feature-major layout the PE contracts over.
   col gathers embgL (feats on partitions 0-63), row gathers embgH (feats on
   partitions 64-127); one vector add merges them into f12^T [128, E].
   This deletes the v1 interleave (2 vector copies) + 8 PE transposes +
   PSUM round-trip + scalar copy per 1024 edges.
2. Edges sharded 100k/core across 8 cores; per core the v1 4-segment int16
   index scheme is kept (dma_gather indices are int16; nodes >= 32768 are
   reached via a statically offset table base per segment).
3. 2048-edge macros (53/core). Tensor stream is 18 matmuls/macro:
   l1: 8 (M=256 in 2 halves x 4 N=512 chunks, K=128)
   l2: 8 (K=256 split x 4 chunks, M=64 zero-padded to 128 so two 512-edge
        chunks stack in one PSUM bank)
   l3: 2 (block-diagonal W3 [128,2] computes two stacked chunks per matmul)
   Scalar: 6 activations; vector: 2 (merge + out bias-add); 1 output DMA.
4. Stage emission is software-pipelined with per-engine skew and PSUM pool
   sizes chosen so every cross-engine wait is pre-satisfied.
"""

import numpy as np

import concourse.bacc as bacc
import concourse.bass as bass
import concourse.mybir as mybir
import concourse.tile as tile
import concourse.tile_sem_assignment as _tsa
from concourse.bass_utils import run_bass_kernel_spmd

# Tile assigns DMASW sem lanes round-robin in scheduled order, while the sim /
# ucode lock each lane to a single SWDGE queue.  With multi-queue gathers the
# blind rotation mixes queues on one lane.  Pin lanes by queue: queue q owns
# lanes {2q, 2q+1} (8 lanes / 4 queues), toggling for pipelining.
_orig_assign_tick = _tsa.TileClockTick._assign_tick


def _queue_affine_assign_tick(self, inst):
    if (
        isinstance(inst, _tsa.DMAInst)
        and getattr(inst, "engine", None) == mybir.EngineType.Pool
        and getattr(inst, "queue_num", None) is not None
    ):
        q = inst.queue_num
        tog = getattr(self, "_q_lane_toggle", None)
        if tog is None:
            tog = self._q_lane_toggle = {}
        t = tog.get(q, 0)
        tog[q] = t ^ 1
        self.next_sw_dma_idx = 2 * q + t
    return _orig_assign_tick(self, inst)


_tsa.TileClockTick._assign_tick = _queue_affine_assign_tick

N_NODES = 50000
N_EDGES = 800000
HID = 64
NCORES = 8
EPC = N_EDGES // NCORES          # edges per core
TILE_E = 512                     # index wrap granularity
SPLIT = 32768                    # int16 index split point
SEG_CAP_TILES = [88, 48, 48, 28]  # 512-tile caps per segment (all % 4 == 0)
T_TOTAL = sum(SEG_CAP_TILES)     # 512-tiles per core
MAC_E = 2048                     # edges per macro (4 x 512-tiles)
N_MACROS = T_TOTAL * TILE_E // MAC_E  # 53

_SEG_BASE = [(0, 0), (0, SPLIT), (SPLIT, 0), (SPLIT, SPLIT)]


def build_nc(repeat: int = 1):
    f32 = mybir.dt.float32
    bf16 = mybir.dt.bfloat16
    i16 = mybir.dt.int16
    relu = mybir.ActivationFunctionType.Relu

    nc = bacc.Bacc("TRN2", target_bir_lowering=False, debug=False,
                   num_swdge_queues=4)

    embgL = nc.dram_tensor("embgL", [N_NODES, 128], bf16, kind="ExternalInput")
    embgH = nc.dram_tensor("embgH", [N_NODES, 128], bf16, kind="ExternalInput")
    colidx = nc.dram_tensor("colidx", [128, T_TOTAL * 32], i16, kind="ExternalInput")
    rowidx = nc.dram_tensor("rowidx", [128, T_TOTAL * 32], i16, kind="ExternalInput")
    w1 = nc.dram_tensor("w1", [128, 256], bf16, kind="ExternalInput")
    w2p = nc.dram_tensor("w2p", [128, 4, 128], bf16, kind="ExternalInput")
    w3b = nc.dram_tensor("w3b", [128, 2], bf16, kind="ExternalInput")
    b1d = nc.dram_tensor("b1", [128, 2], f32, kind="ExternalInput")
    b2d = nc.dram_tensor("b2", [128, 1], f32, kind="ExternalInput")
    b3d = nc.dram_tensor("b3", [2, 1], f32, kind="ExternalInput")
    out = nc.dram_tensor("out", [N_MACROS, 2, 2, 512], f32, kind="ExternalOutput")

    # macro list: (macro_idx, seg)
    macros = []
    t0 = 0
    for s, n in enumerate(SEG_CAP_TILES):
        assert n % 4 == 0
        for m in range(n * TILE_E // MAC_E):
            macros.append((t0 * TILE_E // MAC_E + m, s))
        t0 += n
    assert len(macros) == N_MACROS

    with tile.TileContext(nc) as tc:
        with (
            tc.tile_pool(name="const", bufs=1) as cpool,
            tc.tile_pool(name="gath", bufs=10) as gpool,
            tc.tile_pool(name="act", bufs=2) as apool,
            tc.tile_pool(name="ph1", bufs=2, space="PSUM") as ph1,
            tc.tile_pool(name="ph2", bufs=2, space="PSUM") as ph2,
            tc.tile_pool(name="po3", bufs=1, space="PSUM") as po3,
        ):
            cix = cpool.tile([128, T_TOTAL * 32], i16)
            rix = cpool.tile([128, T_TOTAL * 32], i16)
            w1s = cpool.tile([128, 256], bf16)
            w2s = cpool.tile([128, 4, 128], bf16)
            w3s = cpool.tile([128, 2], bf16)
            b1s = cpool.tile([128, 2], f32)
            b2s = cpool.tile([128, 1], f32)
            b3s = cpool.tile([2, 1], f32)
            nc.sync.dma_start(cix[:], colidx[:])
            nc.sync.dma_start(rix[:], rowidx[:])
            nc.sync.dma_start(w1s[:], w1[:])
            nc.sync.dma_start(w2s[:], w2p[:])
            nc.sync.dma_start(w3s[:], w3b[:])
            nc.sync.dma_start(b1s[:], b1d[:])
            nc.sync.dma_start(b2s[:], b2d[:])
            nc.sync.dma_start(b3s[:], b3d[:])

            state = {}
            qq = [0]

            def st_gather(m):
                mi, s = macros[m]
                cbase, rbase = _SEG_BASE[s]
                ix0 = mi * 128  # 4 tiles x 32 idx-cols
                ccol = gpool.tile([128, 1, MAC_E], bf16, tag="ccol")
                crow = gpool.tile([128, 1, MAC_E], bf16, tag="crow")
                # transpose-mode gathers race when spread across SWDGE queues
                # (completion sems fire early under queue concurrency); a
                # single queue's FIFO is empirically exact.
                nc.gpsimd.dma_gather(
                    ccol[:], embgL[cbase:, :],
                    cix[:, ix0:ix0 + 128], MAC_E, MAC_E, 128,
                    transpose=True, queue_num=0, single_packet=False)
                nc.gpsimd.dma_gather(
                    crow[:], embgH[rbase:, :],
                    rix[:, ix0:ix0 + 128], MAC_E, MAC_E, 128,
                    transpose=True, queue_num=0, single_packet=False)
                state[m] = {"ccol": ccol, "crow": crow}

            def st_merge(m):
                d = state[m]
                f12 = gpool.tile([128, MAC_E], bf16, tag="f12")
                nc.vector.tensor_add(f12[:], d["ccol"][:, 0, :], d["crow"][:, 0, :])
                d["f12"] = f12

            def st_l1(m):
                d = state[m]
                f12 = d["f12"]
                s1a = apool.tile([128, 4, 512], bf16, tag="s1a")
                s1b = apool.tile([128, 4, 512], bf16, tag="s1b")
                for half, (wsl, s1, bcol) in enumerate(
                        [(slice(0, 128), s1a, 0), (slice(128, 256), s1b, 1)]):
                    for cc in range(2):
                        h1 = ph1.tile([128, 2, 512], f32, tag="h1")
                        for j in range(2):
                            c = 2 * cc + j
                            nc.tensor.matmul(h1[:, j, :], w1s[:, wsl],
                                             f12[:, c * 512:(c + 1) * 512],
                                             start=True, stop=True)
                        nc.scalar.activation(
                            s1[:, 2 * cc:2 * cc + 2, :], h1[:], relu,
                            bias=b1s[:, bcol:bcol + 1])
                d["s1a"], d["s1b"] = s1a, s1b

            def st_l2(m):
                d = state[m]
                s1a, s1b = d["s1a"], d["s1b"]
                s2 = apool.tile([128, 2, 512], bf16, tag="s2")
                for p in range(2):
                    h2 = ph2.tile([128, 512], f32, tag="h2")
                    nc.tensor.matmul(h2[:], w2s[:, 0, :], s1a[:, 2 * p, :], start=True, stop=False)
                    nc.tensor.matmul(h2[:], w2s[:, 1, :], s1b[:, 2 * p, :], start=False, stop=False)
                    nc.tensor.matmul(h2[:], w2s[:, 2, :], s1a[:, 2 * p + 1, :], start=False, stop=False)
                    nc.tensor.matmul(h2[:], w2s[:, 3, :], s1b[:, 2 * p + 1, :], start=False, stop=True)
                    nc.scalar.activation(s2[:, p, :], h2[:], relu, bias=b2s[:])
                d["s2"] = s2

            def st_l3(m):
                d = state[m]
                mi, _ = macros[m]
                s2 = d["s2"]
                o3 = po3.tile([2, 2, 512], f32, tag="o3")
                for p in range(2):
                    nc.tensor.matmul(o3[:, p, :], w3s[:], s2[:, p, :], start=True, stop=True)
                stage = apool.tile([2, 2, 512], f32, tag="stage")
                nc.vector.tensor_scalar_add(
                    stage[:].rearrange("p a b -> p (a b)"),
                    o3[:].rearrange("p a b -> p (a b)"), b3s[:])
                nc.sync.dma_start(out[mi, :, :, :], stage[:])
                del state[m]

            # The merge reads gather output 2 steps after the gather issues
            # (one full macro of slack, ~2ms) so the transpose-gather DMA
            # writes land well before the vector engine reads them, on top of
            # the sem wait (the sem can fire early under this runtime).
            # merge(m) is emitted just before l1(m) in the same step so the
            # compute stages keep their measured-best offsets.
            plan = [(4, st_l3), (3, st_l2), (2, st_merge), (2, st_l1), (0, st_gather)]
            nm = N_MACROS
            for _rep in range(repeat):
                for i in range(nm + 4):
                    for si, fn in plan:
                        m = i - si
                        if 0 <= m < nm:
                            fn(m)

    nc.compile()
    return nc


def _wrap16(arr_t512):
    """[T*512] int16 -> [128, T*32] wrapped-by-16 idx layout, replicated x8."""
    T = arr_t512.shape[0] // TILE_E
    a = arr_t512.reshape(T, 32, 16).transpose(2, 0, 1).reshape(16, T * 32)
    return np.tile(a, (8, 1)).astype(np.int16)


def _to_bf16_u16(a32: np.ndarray) -> np.ndarray:
    """f32 -> bf16 (round-to-nearest-even), as uint16."""
    u = np.ascontiguousarray(a32, np.float32).view(np.uint32)
    rounded = (u + 0x7FFF + ((u >> 16) & 1)) >> 16
    return rounded.astype(np.uint16)


def _bf16(a32: np.ndarray):
    import ml_dtypes
    return _to_bf16_u16(np.asarray(a32, np.float32)).view(ml_dtypes.bfloat16)


def prep_inputs(emb, edge_index, W1, b1, W2, b2, W3, b3):
    """Host-side marshalling. Returns (in_maps, origpos_per_core, b3_val)."""
    emb = np.asarray(emb, np.float32)
    ei = np.asarray(edge_index).astype(np.int64)
    W1 = np.asarray(W1, np.float32)
    b1 = np.asarray(b1, np.float32)
    W2 = np.asarray(W2, np.float32)
    b2 = np.asarray(b2, np.float32)
    W3 = np.asarray(W3, np.float32)
    b3 = np.asarray(b3, np.float32)

    embL = np.zeros((N_NODES, 128), np.float32)
    embL[:, 0:64] = emb
    embH = np.zeros((N_NODES, 128), np.float32)
    embH[:, 64:128] = emb
    embgL = _bf16(embL)
    embgH = _bf16(embH)

    w1b = _bf16(W1)                                    # [128, 256]
    # w2p[:, v, :]: v=0: A_k0, v=1: A_k1, v=2: B_k0, v=3: B_k1
    w2pf = np.zeros((128, 4, 128), np.float32)
    w2pf[:, 0, 0:64] = W2[0:128, :]
    w2pf[:, 1, 0:64] = W2[128:256, :]
    w2pf[:, 2, 64:128] = W2[0:128, :]
    w2pf[:, 3, 64:128] = W2[128:256, :]
    w2pb = _bf16(w2pf)
    w3blk = np.zeros((128, 2), np.float32)
    w3blk[0:64, 0] = W3[:, 0]
    w3blk[64:128, 1] = W3[:, 0]
    w3bb = _bf16(w3blk)
    b1p = np.ascontiguousarray(np.stack([b1[0:128], b1[128:256]], axis=1)).astype(np.float32)
    b2p = np.concatenate([b2, b2]).reshape(128, 1).astype(np.float32)
    b3p = np.full((2, 1), float(b3.reshape(-1)[0]), np.float32)

    in_maps = []
    origpos = []
    for c in range(NCORES):
        sl = slice(c * EPC, (c + 1) * EPC)
        col = ei[0, sl]
        row = ei[1, sl]
        seg = (col >= SPLIT) * 2 + (row >= SPLIT)
        cloc = np.zeros(T_TOTAL * TILE_E, np.int16)
        rloc = np.zeros(T_TOTAL * TILE_E, np.int16)
        orig = np.full(T_TOTAL * TILE_E, -1, np.int64)
        off = 0
        for s in range(4):
            msk = np.nonzero(seg == s)[0]
            n = len(msk)
            cap = SEG_CAP_TILES[s] * TILE_E
            assert n <= cap, f"core {c} segment {s}: {n} > cap {cap}"
            cloc[off:off + n] = (col[msk] - _SEG_BASE[s][0]).astype(np.int16)
            rloc[off:off + n] = (row[msk] - _SEG_BASE[s][1]).astype(np.int16)
            orig[off:off + n] = c * EPC + msk
            off += cap
        in_maps.append({
            "embgL": embgL,
            "embgH": embgH,
            "colidx": _wrap16(cloc),
            "rowidx": _wrap16(rloc),
            "w1": w1b,
            "w2p": w2pb,
            "w3b": w3bb,
            "b1": b1p,
            "b2": b2p,
            "b3": b3p,
        })
        origpos.append(orig)
    return in_maps, origpos


def unshard(results, origpos):
    out_full = np.empty((N_EDGES, 1), np.float32)
    for c in range(NCORES):
        raw = results[c]["out"]                         # [53, 2, 2, 512]
        # edge ordinal o in macro: chunk cc = o//512, j = o%512,
        # value at raw[mi, cc%2, cc//2, j]
        vals = raw.transpose(0, 2, 1, 3).reshape(-1)    # [53*2048] in o-order
        orig = origpos[c]
        valid = orig >= 0
        out_full[orig[valid], 0] = vals[valid]
    return out_full


_NC_CACHE = {}


def _get_nc(repeat: int = 1):
    if repeat not in _NC_CACHE:
        _NC_CACHE[repeat] = build_nc(repeat)
    return _NC_CACHE[repeat]


def kernel(**inputs) -> np.ndarray:
    nc = _get_nc(1)
    in_maps, origpos = prep_inputs(
        inputs["emb"], inputs["edge_index"],
        inputs["W1"], inputs["b1"], inputs["W2"], inputs["b2"],
        inputs["W3"], inputs["b3"])
    res = run_bass_kernel_spmd(nc, in_maps, core_ids=list(range(NCORES)))
    return unshard(res.results, origpos)
